# revision 103
# baseline (speedup 1.0000x reference)
# Trainium2 Bass kernel for relative-position causal attention
# (Transformer-XL style: logits = q·k + q·table[n-m], causal softmax, AV, out-proj).
#
# Sharding: tensor-parallel over heads — 16 heads / 8 cores = 2 heads per core.
# Each core computes its heads' projections, attention, and a partial output
# projection [B,D,N] fp16; the host sums the 8 partials in fp32.
#
# Position logits are computed in *diagonal* layout (T[ni, j] = q[nb+ni]·table[j],
# a plain matmul since the table index is the diagonal n-m), then converted to
# row layout with a DMA "shear" through a DRAM scratch strip: partition ni reads
# flat offset ni*(P-1) + c with row pitch P = W+128, which is exactly the
# per-partition-shifted gather no on-chip engine can do. The 128-column pad of
# each strip row is pre-poisoned with -1e30, so the causal mask falls out of the
# pitch arithmetic for free (row ni's reads beyond column n land in the pad).
#
# Precision: every logit-affecting matmul runs as a 3-term bf16 hi/lo split
# (a·b ≈ ah·bh + ah·bl + al·bh, error ~2^-16 relative) instead of true fp32 —
# same accuracy class for this problem (|logit|~8e4 needs absolute error << 1),
# but 3 PE cycles/row instead of 4. fp32r (12.6 effective bits measured on HW)
# is too coarse for the logits. The value path (v, softmax weights, AV,
# out-proj) runs in fp16. The V transpose goes through the DMA XBAR; the
# softmax-weight (P) transposes run on the PE (fp16 identity-matmul into an
# fp16 PSUM staging tile, drained by DVE copies) — the XBAR path cost ~65us
# of serialized DMA-engine time that the PE absorbs for ~29us.
#
# Scheduling (tuned against the TimelineSim cost model, which tracked HW
# within 2% on this kernel):
#  - attention pipeline offsets A(idx) R(-1) B0(-3) B1(-4) C1(-5) C2(-6):
#    the shear read gets 2 slots of DMA lead time, and each block's content
#    adds are split across two slots so the DVE add+rowmax burst of one block
#    doesn't convoy the PE behind a single drain.
#  - the shear round trip is chunk-granular end to end: each 512-col strip
#    READ depends only on the strip WRITE covering its columns, so the
#    diag-matmul -> write -> read -> add chain pipelines at 512-col depth
#    instead of whole-block depth.
#  - the position table (a pure function of for_pos_enc and the fixed sincos
#    basis, no activations) is precomputed on the host like any other weight
#    transform and just DMA-loaded, removing the on-device table projection
#    (~41us PE + 8.4MB of sincos traffic).
#  - PSUM banks (8): A-chunk/transpose/out-proj psums rotate through 3x512
#    slots ("ck"); the content-logit psums use 2x1024 double-banked tiles
#    ("bk") so each DVE add covers 1024 columns in one op (half the psum
#    access-latency count); at2 single-buffered.
#  - DMA queue discipline: strip WRITES + pt2 transposes + pad poisons on the
#    Activation DGE queue, strip READS alone on the SP queue, so the sheared
#    reads never sit behind multi-us writes (worth ~25us).
#  - stages are emitted oldest-first inside each slot to cut PE head-of-line
#    blocking; weight loads go on the scalar queue so the first x tiles on SP
#    aren't serialized behind 4MB of weights at kernel start.

from contextlib import ExitStack

import numpy as np

N = 2048
M = 2048
B = 2
D = 1024
H = 16
DQK = 64
DV = 64
NCORES = 8
HPC = H // NCORES  # heads per core = 2
NZ = N * B
KT = D // 128  # 8 contraction tiles
TW = 2048  # table width (diagonals 0..2047)
NEG = -1.0e30

_cache = {}


def _split_bf16(a):
    """hi = bf16(a), lo = bf16(a - hi), as uint16-viewed bfloat16 arrays."""
    import ml_dtypes

    a = np.asarray(a, dtype=np.float32)
    hi = a.astype(ml_dtypes.bfloat16)
    lo = (a - hi.astype(np.float32)).astype(ml_dtypes.bfloat16)
    return np.ascontiguousarray(hi), np.ascontiguousarray(lo)


def _sincos_rev():
    """sincos basis for diagonals d=0..2047, column-reversed, transposed to
    [D, TW] so sctr[:, jr] = sincos(d=TW-1-jr).  Computed with jax on CPU to
    match the reference's fp32 rounding of inv_freq/phases/sin bitwise."""
    try:
        import jax
        import jax.numpy as jnp

        cpu = jax.devices("cpu")[0]
        with jax.default_device(cpu):
            r = jnp.arange(0.0, float(TW), dtype=jnp.float32)
            inv_freq = 1.0 / (
                10000.0 ** (jnp.arange(0.0, D, 2.0, dtype=jnp.float32) / D)
            )
            phases = r[:, None] * inv_freq[None, :]
            sincos = jnp.concatenate([jnp.sin(phases), jnp.cos(phases)], axis=-1)
            sc = np.asarray(sincos)  # [TW, D]
    except Exception:
        r = np.arange(0.0, float(TW), dtype=np.float32)
        inv_freq = (
            1.0
            / (10000.0 ** (np.arange(0.0, D, 2.0, dtype=np.float32) / np.float32(D)))
        ).astype(np.float32)
        phases = (r[:, None] * inv_freq[None, :]).astype(np.float32)
        sc = np.concatenate(
            [np.sin(phases, dtype=np.float32), np.cos(phases, dtype=np.float32)],
            axis=-1,
        )
    return np.ascontiguousarray(sc[::-1].T.astype(np.float32))  # [D, TW]


USE_DMA_T = True   # DMA XBAR transposes for P and V (else PE transposes)
ABL_NOSHEAR = False  # timing ablation: skip strip write+read (wrong results)
USE_TTR = False    # fused TTR crashes the deployed runtime; use add+reduce_max
SKIP_ATTN = False  # debug: bypass phase_attn (attn16 <- memset)
SKIP_AV = False    # debug: run attn up to ptT, skip AV matmuls + attn copy
SKIP_SHEAR = False # debug: skip strip write/read; Lp <- memset
STOP_AT = 9        # debug: 1=diag only, 2=+content/TTR, 3=+exp/mul, 4=+transpose, 9=full
SEQ_HEADS = False  # debug: serialize heads, baseline-style per-head AV psum


def _build(loop=1, dbg=False):
    import concourse.bacc as bacc
    import concourse.mybir as mybir
    import concourse.tile as tile
    from concourse.bass import AP
    from concourse.tile_rust import add_dep_helper

    f32 = mybir.dt.float32
    f16 = mybir.dt.float16
    bf16 = mybir.dt.bfloat16
    AX = mybir.AxisListType.X
    MAX = mybir.AluOpType.max
    ADD = mybir.AluOpType.add
    SUB = mybir.AluOpType.subtract
    EXP = mybir.ActivationFunctionType.Exp

    nc = bacc.Bacc("TRN2", target_bir_lowering=False, debug=False, num_devices=NCORES)

    xqh = nc.dram_tensor("xqh", [D, N], bf16, kind="ExternalInput")
    xql = nc.dram_tensor("xql", [D, N], bf16, kind="ExternalInput")
    xkh = nc.dram_tensor("xkh", [D, N], bf16, kind="ExternalInput")
    xkl = nc.dram_tensor("xkl", [D, N], bf16, kind="ExternalInput")
    tbh = nc.dram_tensor("tbh", [128, 2 * TW], bf16, kind="ExternalInput")
    tbl = nc.dram_tensor("tbl", [128, 2 * TW], bf16, kind="ExternalInput")
    wqh = nc.dram_tensor("wqh", [D, 256], bf16, kind="ExternalInput")
    wql = nc.dram_tensor("wql", [D, 256], bf16, kind="ExternalInput")
    wkh = nc.dram_tensor("wkh", [D, 256], bf16, kind="ExternalInput")
    wkl = nc.dram_tensor("wkl", [D, 256], bf16, kind="ExternalInput")
    wv = nc.dram_tensor("wv", [D, 256], bf16, kind="ExternalInput")
    wo = nc.dram_tensor("wo", [256, D], f16, kind="ExternalInput")
    outT = nc.dram_tensor("outT", [D, N], f16, kind="ExternalOutput")
    dbg_t = {}
    if dbg:
        for nm, shp, dt in [
            ("d_qh", [128, NZ], bf16),
            ("d_ql", [128, NZ], bf16),
            ("d_kh", [128, NZ], bf16),
            ("d_th", [128, 2 * TW], bf16),
            ("d_V", [128, B * 16 * 128], f16),
            ("d_L15", [128, 2048], f32),
            ("d_P15", [128, 2048], f16),
            ("d_pt15", [128, 16 * 128], f16),
            ("d_attn16", [128, NZ], f16),
        ]:
            dbg_t[nm] = nc.dram_tensor(nm, shp, dt, kind="ExternalOutput")

    # DRAM scratch strips for the diagonal->row shear, one per (z, i) holding
    # BOTH heads: row layout [h0 data W][pad 128][h1 data W][pad 128], pitch
    # P2 = 2W + 256. The pads are poisoned with -1e30 once, outside the timing
    # loop; the sheared read of row ni lands in the pad exactly for the masked
    # (m > n) columns, so causality needs no separate masking pass.
    scr = {}
    for hp in range(2):
        for i in range(16):
            W = 128 * (i + 1)
            P2 = 2 * W + 256
            scr[(hp, i)] = nc.dram_tensor(
                f"scr_{hp}_{i}", [128 * P2], f32, kind="Internal"
            )

    with tile.TileContext(nc) as tc:
        with ExitStack() as ctx:
            wpool = ctx.enter_context(tc.tile_pool(name="wpool", bufs=1))
            big = ctx.enter_context(tc.tile_pool(name="big", bufs=1))
            work = ctx.enter_context(tc.tile_pool(name="work", bufs=3))

            # ---- weights to SBUF ----
            w_sb = {}
            for nm, dram, dt in [
                ("qh", wqh, bf16),
                ("ql", wql, bf16),
                ("kh", wkh, bf16),
                ("kl", wkl, bf16),
                ("v", wv, bf16),
            ]:
                t = wpool.tile([128, KT, 256], dt, tag=f"w{nm}", name=f"w{nm}")
                nc.scalar.dma_start(t[:], dram.ap().rearrange("(t p) m -> p t m", p=128))
                w_sb[nm] = t
            wo_sb = wpool.tile([128, 2, D], f16)
            nc.scalar.dma_start(
                wo_sb[:], wo.ap().rearrange("(g p) d -> p g d", p=128)
            )

            poison = wpool.tile([128, 256], f32)
            nc.vector.memset(poison[:], NEG)
            from concourse.masks import make_identity

            ident16 = wpool.tile([128, 128], f16)
            make_identity(nc, ident16[:])

            # ---- poison pads: once per launch, outside the timing loop ----
            pad_writes = {}
            for hp in range(2):
                for i in range(16):
                    W = 128 * (i + 1)
                    P2 = 2 * W + 256
                    pad_writes[(hp, i)] = nc.scalar.dma_start(
                        AP(scr[(hp, i)], W, [[P2, 128], [W + 128, 2], [1, 128]]),
                        poison[:].rearrange("p (a b) -> p a b", a=2),
                    )

            # ---- persistent activations ----
            # this core owns ONE batch element z and FOUR heads = 2 head-pairs;
            # hp indexes the pair, columns are hp-major: col = hp*N + n
            q_hi = big.tile([128, NZ], bf16)  # [pair-local hd, hp*N+n]
            q_lo = big.tile([128, NZ], bf16)
            k_hi = big.tile([128, NZ], bf16)
            k_lo = big.tile([128, NZ], bf16)
            t_hi = big.tile([128, 2, TW], bf16)  # reversed diag table per pair
            t_lo = big.tile([128, 2, TW], bf16)
            V_all = big.tile([128, 2 * 16, 128], f16)  # [m-part, hp*16+mb, hv]
            attn16 = big.tile([128, NZ], f16)  # [hv, hp*N+n], normalized

            def split_out(pp_t, hi_sb, lo_sb, sl):
                """psum fp32 chunk -> hi (bf16, ACT) and lo = x - hi (bf16, DVE)."""
                nc.scalar.copy(hi_sb[:, sl], pp_t[:])
                nc.vector.tensor_tensor(
                    out=lo_sb[:, sl], in0=pp_t[:], in1=hi_sb[:, sl], op=SUB
                )

            def phase_proj():
                with tc.tile_pool(name="pp", bufs=1, space="PSUM") as pp, tc.tile_pool(
                    name="xp", bufs=3
                ) as xp:

                    # --- q projection: one pass, 2 pair-tiles x 4 chunks ---
                    psq = [
                        pp.tile([128, 512], f32, tag=f"pj{g}{c}", name=f"q{g}{c}")
                        for g in range(2)
                        for c in range(4)
                    ]
                    for t in range(KT):
                        xh_t = xp.tile([128, 2048], bf16, tag="xh")
                        xl_t = xp.tile([128, 2048], bf16, tag="xl")
                        nc.sync.dma_start(
                            xh_t[:],
                            xqh.ap().rearrange("(t p) n -> p t n", p=128)[:, t, :],
                        )
                        nc.sync.dma_start(
                            xl_t[:],
                            xql.ap().rearrange("(t p) n -> p t n", p=128)[:, t, :],
                        )
                        for g in range(2):
                            ws = slice(128 * g, 128 * (g + 1))
                            for c in range(4):
                                rh = xh_t[:, 512 * c : 512 * (c + 1)]
                                rl = xl_t[:, 512 * c : 512 * (c + 1)]
                                ps = psq[4 * g + c]
                                nc.tensor.matmul(
                                    ps[:], w_sb["qh"][:, t, ws], rh,
                                    start=(t == 0), stop=False,
                                )
                                nc.tensor.matmul(
                                    ps[:], w_sb["qh"][:, t, ws], rl,
                                    start=False, stop=False,
                                )
                                nc.tensor.matmul(
                                    ps[:], w_sb["ql"][:, t, ws], rh,
                                    start=False, stop=(t == KT - 1),
                                )
                    for g in range(2):
                        for c in range(4):
                            split_out(
                                psq[4 * g + c], q_hi, q_lo,
                                slice(g * N + 512 * c, g * N + 512 * (c + 1)),
                            )

                    # table loads: after the weight/q traffic is queued, well
                    # before stage_a(0) needs them
                    nc.scalar.dma_start(
                        t_hi[:].rearrange("p a b -> p (a b)"), tbh.ap()
                    )
                    nc.scalar.dma_start(
                        t_lo[:].rearrange("p a b -> p (a b)"), tbl.ap()
                    )

                    # --- k+v projection: 1024-col half passes, (2k+2v)x2 chunks ---
                    vT16 = xp.tile([128, NZ], f16, tag="vT")
                    for half in range(2):
                        cs = 1024 * half
                        psk = [
                            pp.tile([128, 512], f32, tag=f"pj{g}{c}", name=f"k{g}{c}")
                            for g in range(2)
                            for c in range(2)
                        ]
                        psv = [
                            pp.tile([128, 512], f32, tag=f"pj{g}{c + 2}", name=f"v{g}{c}")
                            for g in range(2)
                            for c in range(2)
                        ]
                        for t in range(KT):
                            xh_t = xp.tile([128, 1024], bf16, tag="xh2")
                            xl_t = xp.tile([128, 1024], bf16, tag="xl2")
                            nc.sync.dma_start(
                                xh_t[:],
                                xkh.ap().rearrange("(t p) n -> p t n", p=128)[
                                    :, t, cs : cs + 1024
                                ],
                            )
                            nc.sync.dma_start(
                                xl_t[:],
                                xkl.ap().rearrange("(t p) n -> p t n", p=128)[
                                    :, t, cs : cs + 1024
                                ],
                            )
                            for g in range(2):
                                ws = slice(128 * g, 128 * (g + 1))
                                for c in range(2):
                                    rh = xh_t[:, 512 * c : 512 * (c + 1)]
                                    rl = xl_t[:, 512 * c : 512 * (c + 1)]
                                    ps = psk[2 * g + c]
                                    nc.tensor.matmul(
                                        ps[:], w_sb["kh"][:, t, ws], rh,
                                        start=(t == 0), stop=False,
                                    )
                                    nc.tensor.matmul(
                                        ps[:], w_sb["kh"][:, t, ws], rl,
                                        start=False, stop=False,
                                    )
                                    nc.tensor.matmul(
                                        ps[:], w_sb["kl"][:, t, ws], rh,
                                        start=False, stop=(t == KT - 1),
                                    )
                                    nc.tensor.matmul(
                                        psv[2 * g + c][:], w_sb["v"][:, t, ws], rh,
                                        start=(t == 0), stop=(t == KT - 1),
                                    )
                        for g in range(2):
                            for c in range(2):
                                sl = slice(
                                    g * N + cs + 512 * c, g * N + cs + 512 * (c + 1)
                                )
                                split_out(psk[2 * g + c], k_hi, k_lo, sl)
                                nc.scalar.copy(vT16[:, sl], psv[2 * g + c][:])


                    # --- V tiles [m, hv] via one DMA XBAR transpose ---
                    nc.sync.dma_start_transpose(
                        V_all[:].rearrange("p a b -> p (a b)").rearrange(
                            "p (a b) -> p a b", b=128
                        ),
                        vT16[:],
                    )

            def phase_attn():
                # Software-pipelined over the 32 (z, i) blocks, depth 5:
                #   A(b):   diag matmuls -> per-chunk tT copies (ACT) + strip writes
                #   R(b):   sheared strip read (one 3D-AP DMA, both heads)
                #   B(b):   content matmuls -> in-place add (DVE) -> rowmax
                #   C1(b):  exp (ACT, accum sums) -> recip -> normalize -> ptT DMA
                #   C2(b):  column-packed AV -> attn16 copy; out-proj every 4th
                # Per-engine instruction streams execute in order, so emitting
                # A(i) R(i-1) B(i-2) C1(i-3) C2(i-4) gives each DMA round trip
                # a full pipeline iteration to complete.
                blocks = [(hp, i) for i in range(16) for hp in range(2)]
                NB = len(blocks)
                state = {}
                quarter_left = [8, 8, 8, 8]

                def stage_a(b):
                    hp, i = blocks[b]
                    nb = 128 * i
                    W = nb + 128
                    P2 = 2 * W + 256
                    nchunks = (W + 511) // 512
                    st = state[b] = {}
                    q_b = [
                        (q_hi[64 * h : 64 * (h + 1), hp * N + nb : hp * N + W],
                         q_lo[64 * h : 64 * (h + 1), hp * N + nb : hp * N + W])
                        for h in range(2)
                    ]
                    st["q_b"] = q_b
                    s = scr[(hp, i)]
                    w_insts = []
                    for bc in range(nchunks):
                        wdt = min(512, W - 512 * bc)
                        tps = [
                            pck.tile([128, 512], f32, tag="ck", name=f"tps{h}")
                            for h in range(2)
                        ]
                        for h in range(2):
                            hs = slice(64 * h, 64 * (h + 1))
                            th_c = t_hi[hs, hp, TW - W + 512 * bc : TW - W + 512 * bc + wdt]
                            tl_c = t_lo[hs, hp, TW - W + 512 * bc : TW - W + 512 * bc + wdt]
                            qh_b, ql_b = q_b[h]
                            nc.tensor.matmul(
                                tps[h][:, :wdt], qh_b, th_c, start=True, stop=False
                            )
                            nc.tensor.matmul(
                                tps[h][:, :wdt], qh_b, tl_c, start=False, stop=False
                            )
                            nc.tensor.matmul(
                                tps[h][:, :wdt], ql_b, th_c, start=False, stop=True
                            )
                        tTc = tpo.tile([128, 2, 512], f32, tag="tTc", name="tTc")
                        for h in range(2):
                            if (bc + h) % 2 == 0:
                                nc.scalar.copy(tTc[:, h, :wdt], tps[h][:, :wdt])
                            else:
                                nc.vector.tensor_copy(tTc[:, h, :wdt], tps[h][:, :wdt])
                        if not ABL_NOSHEAR:
                            eng = nc.scalar
                            w_insts.append(
                                eng.dma_start(
                                    AP(s, 512 * bc, [[P2, 128], [W + 128, 2], [1, wdt]]),
                                    tTc[:, :, :wdt],
                                )
                            )
                        else:
                            sink = work.tile([128, 1], f32, tag="sink", name="sink")
                            nc.vector.reduce_max(out=sink[:], in_=tTc[:, 0, :4], axis=AX)
                    st["w_insts"] = w_insts

                def stage_r(b):
                    hp, i = blocks[b]
                    W = 128 * (i + 1)
                    P2 = 2 * W + 256
                    st = state[b]
                    s = scr[(hp, i)]
                    Lp2 = lpo.tile([128, 2, 2048], f32, tag="Lp2", name="Lp2")
                    if ABL_NOSHEAR:
                        nc.vector.memset(Lp2[:, :, :W], -1.0)
                    else:
                        nch = (W + 511) // 512
                        ranges = [
                            (512 * c, min(512 * (c + 1), W)) for c in range(nch)
                        ]
                        for c0, c1 in ranges:
                            r_inst = nc.sync.dma_start(
                                Lp2[:, :, c0:c1],
                                AP(
                                    s,
                                    127 + c0,
                                    [[P2 - 1, 128], [W + 128, 2], [1, c1 - c0]],
                                ),
                            )
                            # depend only on the strip writes covering [c0, c1)
                            for wc, w in enumerate(st["w_insts"]):
                                if 512 * wc < c1 and 512 * (wc + 1) > c0:
                                    add_dep_helper(
                                        r_inst.ins, w.ins,
                                        reason="shear read after strip write",
                                    )
                            add_dep_helper(
                                r_inst.ins, pad_writes[(hp, i)].ins,
                                reason="shear read after poison pad",
                            )
                    st["Lp2"] = Lp2

                def stage_b(b, half):
                    hp, i = blocks[b]
                    W = 128 * (i + 1)
                    npair = (W + 1023) // 1024
                    st = state[b]
                    Lp2 = st["Lp2"]
                    q_b = st["q_b"]
                    plo = 0 if half == 0 else (npair + 1) // 2
                    phi = (npair + 1) // 2 if half == 0 else npair
                    for p_ in range(plo, phi):
                        pw = min(1024, W - 1024 * p_)
                        cps = [
                            pbk.tile([128, 1024], f32, tag="bk", name=f"cps{h}")
                            for h in range(2)
                        ]
                        for h in range(2):
                            hs = slice(64 * h, 64 * (h + 1))
                            qh_b, ql_b = q_b[h]
                            for cc in range(0, pw, 512):
                                wdt = min(512, pw - cc)
                                c0 = 1024 * p_ + cc
                                kh_c = k_hi[hs, hp * N + c0 : hp * N + c0 + wdt]
                                kl_c = k_lo[hs, hp * N + c0 : hp * N + c0 + wdt]
                                dstp = cps[h][:, cc : cc + wdt]
                                nc.tensor.matmul(
                                    dstp, qh_b, kh_c, start=True, stop=False,
                                    skip_group_check=True,
                                )
                                nc.tensor.matmul(
                                    dstp, qh_b, kl_c, start=False, stop=False,
                                    skip_group_check=True,
                                )
                                nc.tensor.matmul(
                                    dstp, ql_b, kh_c, start=False, stop=True,
                                    skip_group_check=True,
                                )
                        for h in range(2):
                            sl = Lp2[:, h, 1024 * p_ : 1024 * p_ + pw]
                            nc.vector.tensor_add(sl, cps[h][:, :pw], sl)
                    if half == 1:
                        negmax = work.tile([128, 2], f32, tag="ngm", name="ngm")
                        for h in range(2):
                            nc.vector.tensor_reduce(
                                out=negmax[:, h : h + 1],
                                in_=Lp2[:, h, :W],
                                axis=AX,
                                op=MAX,
                                negate=True,
                            )
                        st["negmax"] = negmax

                def stage_c1(b):
                    hp, i = blocks[b]
                    W = 128 * (i + 1)
                    st = state[b]
                    Lp2 = st["Lp2"]
                    negmax = st["negmax"]
                    P2_sb = ppo.tile([128, 4096], f16, tag="P2", name="P2")
                    ssum = work.tile([128, 2], f32, tag="ssum", name="ssum")
                    for h in range(2):
                        nc.scalar.activation(
                            P2_sb[:, W * h : W * (h + 1)],
                            Lp2[:, h, :W],
                            EXP,
                            bias=negmax[:, h : h + 1],
                            scale=1.0,
                            accum_out=ssum[:, h : h + 1],
                        )
                    rsum = work.tile([128, 2], f32, tag="rsum", name="rsum")
                    nc.vector.reciprocal(rsum[:], ssum[:])
                    pt2 = pto.tile([128, 32, 128], f16, tag="pt2", name="pt2")
                    for h in range(2):
                        nc.vector.tensor_scalar_mul(
                            P2_sb[:, W * h : W * (h + 1)],
                            P2_sb[:, W * h : W * (h + 1)],
                            rsum[:, h : h + 1],
                        )
                    ntile = 2 * (i + 1)
                    for g0 in range(0, ntile, 4):
                        gn = min(4, ntile - g0)
                        tp_ps = pck.tile([128, 512], f16, tag="ck", name="tp_ps")
                        for j in range(gn):
                            nc.tensor.transpose(
                                tp_ps[:, 128 * j : 128 * (j + 1)],
                                P2_sb[:, 128 * (g0 + j) : 128 * (g0 + j + 1)],
                                ident16[:],
                            )
                        dst = pt2[:, g0 : g0 + gn, :].rearrange("p a b -> p (a b)")
                        if (g0 // 4) % 3 == 0:
                            nc.vector.tensor_copy(dst, tp_ps[:, : 128 * gn])
                        else:
                            nc.scalar.copy(dst, tp_ps[:, : 128 * gn])
                    st["pt2"] = pt2
                    if dbg and hp == 0 and i == 15:
                        nc.sync.dma_start(dbg_t["d_L15"].ap(), Lp2[:, 0:1, :].rearrange("p a b -> p (a b)"))
                        nc.sync.dma_start(dbg_t["d_P15"].ap(), P2_sb[:, :2048])
                        nc.sync.dma_start(
                            dbg_t["d_pt15"].ap(),
                            pt2[:, :16, :].rearrange("p a b -> p (a b)"),
                        )

                def stage_c2(b, pa1, oso):
                    hp, i = blocks[b]
                    nb = 128 * i
                    st = state.pop(b)
                    pt2 = st["pt2"]
                    at2 = pa1.tile([128, 128], f32, tag="at2", name="at2")
                    for h in range(2):
                        hs = slice(64 * h, 64 * (h + 1))
                        for mt in range(i + 1):
                            nc.tensor.matmul(
                                at2[hs, :],
                                V_all[:, hp * 16 + mt, hs],
                                pt2[:, (i + 1) * h + mt, :],
                                start=(mt == 0),
                                stop=(mt == i),
                                skip_group_check=True,
                            )
                    nc.vector.tensor_copy(
                        attn16[:, hp * N + nb : hp * N + nb + 128], at2[:]
                    )
                    # out-projection for an n-quarter once all 8 of its blocks
                    # (4 i-values x 2 hp) have passed AV
                    quarter_left[i // 4] -= 1
                    if quarter_left[i // 4] == 0:
                        nn = i // 4
                        for dh in range(2):
                            o_sb = oso.tile([128, 4, 512], f16, tag="osb", name="osb")
                            for dq in range(4):
                                dc = 4 * dh + dq
                                o_ps = pck.tile([128, 512], f32, tag="ck", name="ops")
                                for g in range(2):
                                    nc.tensor.matmul(
                                        o_ps[:],
                                        wo_sb[:, g, 128 * dc : 128 * (dc + 1)],
                                        attn16[:, g * N + 512 * nn : g * N + 512 * (nn + 1)],
                                        start=(g == 0),
                                        stop=(g == 1),
                                    )
                                if dc % 2 == 0:
                                    nc.scalar.copy(o_sb[:, dq, :], o_ps[:])
                                else:
                                    nc.vector.tensor_copy(o_sb[:, dq, :], o_ps[:])
                            nc.sync.dma_start(
                                AP(
                                    outT,
                                    512 * nn + dh * 4 * 128 * N,
                                    [[N, 128], [128 * N, 4], [1, 512]],
                                ),
                                o_sb[:],
                            )

                with tc.tile_pool(name="pa1", bufs=1, space="PSUM") as pa1, tc.tile_pool(
                    name="pbk", bufs=2, space="PSUM"
                ) as pbk, tc.tile_pool(
                    name="pck", bufs=3, space="PSUM"
                ) as pck, tc.tile_pool(name="tpo", bufs=3) as tpo, tc.tile_pool(
                    name="lpo", bufs=4
                ) as lpo, tc.tile_pool(name="ppo", bufs=2) as ppo, tc.tile_pool(
                    name="pto", bufs=2
                ) as pto, tc.tile_pool(name="oso", bufs=2) as oso:
                    if dbg:
                        nc.sync.dma_start(dbg_t["d_qh"].ap(), q_hi[:])
                        nc.sync.dma_start(dbg_t["d_ql"].ap(), q_lo[:])
                        nc.sync.dma_start(dbg_t["d_kh"].ap(), k_hi[:])
                        nc.sync.dma_start(dbg_t["d_th"].ap(), t_hi[:].rearrange("p a b -> p (a b)"))
                        nc.sync.dma_start(
                            dbg_t["d_V"].ap(), V_all[:].rearrange("p a b -> p (a b)")
                        )
                    for idx in range(NB + 6):
                        if 1 <= idx < NB + 1:
                            stage_r(idx - 1)
                        if 6 <= idx:
                            stage_c2(idx - 6, pa1, oso)
                        if 5 <= idx < NB + 5:
                            stage_c1(idx - 5)
                        if 4 <= idx < NB + 4:
                            stage_b(idx - 4, 1)
                        if 3 <= idx < NB + 3:
                            stage_b(idx - 3, 0)
                        if idx < NB:
                            stage_a(idx)
                    if dbg:
                        nc.sync.dma_start(dbg_t["d_attn16"].ap(), attn16[:])

            def loop_body():
                phase_proj()
                phase_attn()

            if loop == 1:
                loop_body()
            else:
                with tc.For_i(0, loop, 1):
                    loop_body()

    nc.compile()
    return nc


def _prep_inputs(x_q, x_kv, to_q, to_kv, for_pos_enc, to_o):
    # core c owns batch element z = c // 4 and heads [4*(c%4), 4*(c%4)+4)
    xq = np.asarray(x_q, dtype=np.float32).transpose(2, 1, 0)  # [D, B, N]
    xkv = np.asarray(x_kv, dtype=np.float32).transpose(2, 1, 0)
    xq_s = [_split_bf16(np.ascontiguousarray(xq[:, z, :])) for z in range(B)]
    xk_s = [_split_bf16(np.ascontiguousarray(xkv[:, z, :])) for z in range(B)]
    if "sctr" not in _cache:
        _cache["sctr"] = _sincos_rev()  # [D, TW] fp32, reversed columns
    sctr = _cache["sctr"]
    to_q = np.asarray(to_q, dtype=np.float32)
    to_kv = np.asarray(to_kv, dtype=np.float32)
    fpe = np.asarray(for_pos_enc, dtype=np.float32)
    to_o = np.asarray(to_o, dtype=np.float32)
    # host-precomputed position table (pure function of the weights):
    # tb_all[h*DQK+d, j] = fpe[h, d, :] . sctr[:, j]
    tb_all = fpe.reshape(H * DQK, D) @ sctr  # [1024, TW] fp32
    in_maps = []
    for c in range(NCORES):
        z, hg = c // 4, c % 4
        hs = slice(4 * hg, 4 * (hg + 1))
        wq = np.ascontiguousarray(to_q[hs].reshape(4 * DQK, D).T)
        wk = np.ascontiguousarray(to_kv[hs, :DQK].reshape(4 * DQK, D).T)
        wvc = np.ascontiguousarray(to_kv[hs, DQK:].reshape(4 * DV, D).T)
        woc = np.ascontiguousarray(to_o[:, hs, :].reshape(D, 4 * DV).T).astype(
            np.float16
        )
        wq_h, wq_l = _split_bf16(wq)
        wk_h, wk_l = _split_bf16(wk)
        wv_h, _ = _split_bf16(wvc)
        # per-core table in the device tile layout [p=pair-local hd, g=pair, j]
        t_core = tb_all[256 * hg : 256 * (hg + 1)].reshape(2, 128, TW)
        t_core = np.ascontiguousarray(t_core.transpose(1, 0, 2).reshape(128, 2 * TW))
        tb_h, tb_l = _split_bf16(t_core)
        in_maps.append(
            {
                "xqh": xq_s[z][0], "xql": xq_s[z][1],
                "xkh": xk_s[z][0], "xkl": xk_s[z][1],
                "tbh": tb_h, "tbl": tb_l,
                "wqh": wq_h, "wql": wq_l,
                "wkh": wk_h, "wkl": wk_l,
                "wv": wv_h, "wo": woc,
            }
        )
    return in_maps


def kernel(x_q, x_kv, to_q, to_kv, for_pos_enc, to_o):
    from concourse.bass_utils import run_bass_kernel_spmd

    if "nc" not in _cache:
        _cache["nc"] = _build()
    nc = _cache["nc"]
    in_maps = _prep_inputs(x_q, x_kv, to_q, to_kv, for_pos_enc, to_o)
    res = run_bass_kernel_spmd(nc, in_maps, core_ids=list(range(NCORES)))
    acc = np.zeros((B, D, N), dtype=np.float32)
    for c in range(NCORES):
        acc[c // 4] += res.results[c]["outT"].astype(np.float32)
    return np.ascontiguousarray(acc.transpose(2, 0, 1)).astype(np.float32)



# revision 110
# speedup vs baseline: 5.1952x; 5.1952x over previous
# Trainium2 Bass kernel for relative-position causal attention
# (Transformer-XL style: logits = q·k + q·table[n-m], causal softmax, AV, out-proj).
#
# Sharding: tensor-parallel over heads — 16 heads / 8 cores = 2 heads per core.
# Each core computes its heads' projections, attention, and a partial output
# projection [B,D,N] fp16; the host sums the 8 partials in fp32.
#
# Position logits are computed in *diagonal* layout (T[ni, j] = q[nb+ni]·table[j],
# a plain matmul since the table index is the diagonal n-m), then converted to
# row layout with a DMA "shear" through a DRAM scratch strip: partition ni reads
# flat offset ni*(P-1) + c with row pitch P = W+128, which is exactly the
# per-partition-shifted gather no on-chip engine can do. The 128-column pad of
# each strip row is pre-poisoned with -1e30, so the causal mask falls out of the
# pitch arithmetic for free (row ni's reads beyond column n land in the pad).
#
# Precision: every logit-affecting matmul runs as a 3-term bf16 hi/lo split
# (a·b ≈ ah·bh + ah·bl + al·bh, error ~2^-16 relative) instead of true fp32 —
# same accuracy class for this problem (|logit|~8e4 needs absolute error << 1),
# but 3 PE cycles/row instead of 4. fp32r (12.6 effective bits measured on HW)
# is too coarse for the logits. The value path (v, softmax weights, AV,
# out-proj) runs in fp16. The V transpose goes through the DMA XBAR; the
# softmax-weight (P) transposes run on the PE (fp16 identity-matmul into an
# fp16 PSUM staging tile, drained by DVE copies) — the XBAR path cost ~65us
# of serialized DMA-engine time that the PE absorbs for ~29us.
#
# Scheduling (tuned against the TimelineSim cost model, which tracked HW
# within 2% on this kernel):
#  - attention pipeline offsets A(idx) R(-1) B0(-3) B1(-4) C1(-5) C2(-6):
#    the shear read gets 2 slots of DMA lead time, and each block's content
#    adds are split across two slots so the DVE add+rowmax burst of one block
#    doesn't convoy the PE behind a single drain.
#  - the shear round trip is chunk-granular end to end: each 512-col strip
#    READ depends only on the strip WRITE covering its columns, so the
#    diag-matmul -> write -> read -> add chain pipelines at 512-col depth
#    instead of whole-block depth.
#  - the position table (a pure function of for_pos_enc and the fixed sincos
#    basis, no activations) is precomputed on the host like any other weight
#    transform and just DMA-loaded, removing the on-device table projection
#    (~41us PE + 8.4MB of sincos traffic).
#  - PSUM banks (8): A-chunk/transpose/out-proj psums rotate through 3x512
#    slots ("ck"); the content-logit psums use 2x1024 double-banked tiles
#    ("bk") so each DVE add covers 1024 columns in one op (half the psum
#    access-latency count); at2 single-buffered.
#  - DMA queue discipline: strip WRITES + pad poisons on the Activation DGE
#    queue, strip READS alone on the SP queue, so the sheared reads never sit
#    behind multi-us writes (worth ~25us).
#  - staging depths: tTc (strip-write staging) triple-buffered, out-proj
#    staging double-buffered; P-transpose drains split 2/3 ACT 1/3 DVE —
#    these re-tunings were each re-measured as earlier constraints lifted.
#  - stages are emitted oldest-first inside each slot to cut PE head-of-line
#    blocking; weight loads go on the scalar queue so the first x tiles on SP
#    aren't serialized behind 4MB of weights at kernel start.

from contextlib import ExitStack

import numpy as np

N = 2048
M = 2048
B = 2
D = 1024
H = 16
DQK = 64
DV = 64
NCORES = 8
HPC = H // NCORES  # heads per core = 2
NZ = N * B
KT = D // 128  # 8 contraction tiles
TW = 2048  # table width (diagonals 0..2047)
NEG = -1.0e30

_cache = {}


def _split_bf16(a):
    """hi = bf16(a), lo = bf16(a - hi), as uint16-viewed bfloat16 arrays."""
    import ml_dtypes

    a = np.asarray(a, dtype=np.float32)
    hi = a.astype(ml_dtypes.bfloat16)
    lo = (a - hi.astype(np.float32)).astype(ml_dtypes.bfloat16)
    return np.ascontiguousarray(hi), np.ascontiguousarray(lo)


def _sincos_rev():
    """sincos basis for diagonals d=0..2047, column-reversed, transposed to
    [D, TW] so sctr[:, jr] = sincos(d=TW-1-jr).  Computed with jax on CPU to
    match the reference's fp32 rounding of inv_freq/phases/sin bitwise."""
    try:
        import jax
        import jax.numpy as jnp

        cpu = jax.devices("cpu")[0]
        with jax.default_device(cpu):
            r = jnp.arange(0.0, float(TW), dtype=jnp.float32)
            inv_freq = 1.0 / (
                10000.0 ** (jnp.arange(0.0, D, 2.0, dtype=jnp.float32) / D)
            )
            phases = r[:, None] * inv_freq[None, :]
            sincos = jnp.concatenate([jnp.sin(phases), jnp.cos(phases)], axis=-1)
            sc = np.asarray(sincos)  # [TW, D]
    except Exception:
        r = np.arange(0.0, float(TW), dtype=np.float32)
        inv_freq = (
            1.0
            / (10000.0 ** (np.arange(0.0, D, 2.0, dtype=np.float32) / np.float32(D)))
        ).astype(np.float32)
        phases = (r[:, None] * inv_freq[None, :]).astype(np.float32)
        sc = np.concatenate(
            [np.sin(phases, dtype=np.float32), np.cos(phases, dtype=np.float32)],
            axis=-1,
        )
    return np.ascontiguousarray(sc[::-1].T.astype(np.float32))  # [D, TW]


USE_DMA_T = True   # DMA XBAR transposes for P and V (else PE transposes)
ABL_NOSHEAR = False  # timing ablation: skip strip write+read (wrong results)
USE_TTR = False    # fused TTR crashes the deployed runtime; use add+reduce_max
SKIP_ATTN = False  # debug: bypass phase_attn (attn16 <- memset)
SKIP_AV = False    # debug: run attn up to ptT, skip AV matmuls + attn copy
SKIP_SHEAR = False # debug: skip strip write/read; Lp <- memset
STOP_AT = 9        # debug: 1=diag only, 2=+content/TTR, 3=+exp/mul, 4=+transpose, 9=full
SEQ_HEADS = False  # debug: serialize heads, baseline-style per-head AV psum


def _build(loop=1, dbg=False):
    import concourse.bacc as bacc
    import concourse.mybir as mybir
    import concourse.tile as tile
    from concourse.bass import AP
    from concourse.tile_rust import add_dep_helper

    f32 = mybir.dt.float32
    f16 = mybir.dt.float16
    bf16 = mybir.dt.bfloat16
    AX = mybir.AxisListType.X
    MAX = mybir.AluOpType.max
    ADD = mybir.AluOpType.add
    SUB = mybir.AluOpType.subtract
    EXP = mybir.ActivationFunctionType.Exp

    nc = bacc.Bacc("TRN2", target_bir_lowering=False, debug=False, num_devices=NCORES)

    xqh = nc.dram_tensor("xqh", [D, N], bf16, kind="ExternalInput")
    xql = nc.dram_tensor("xql", [D, N], bf16, kind="ExternalInput")
    xkh = nc.dram_tensor("xkh", [D, N], bf16, kind="ExternalInput")
    xkl = nc.dram_tensor("xkl", [D, N], bf16, kind="ExternalInput")
    tbh = nc.dram_tensor("tbh", [128, 2 * TW], bf16, kind="ExternalInput")
    tbl = nc.dram_tensor("tbl", [128, 2 * TW], bf16, kind="ExternalInput")
    wqh = nc.dram_tensor("wqh", [D, 256], bf16, kind="ExternalInput")
    wql = nc.dram_tensor("wql", [D, 256], bf16, kind="ExternalInput")
    wkh = nc.dram_tensor("wkh", [D, 256], bf16, kind="ExternalInput")
    wkl = nc.dram_tensor("wkl", [D, 256], bf16, kind="ExternalInput")
    wv = nc.dram_tensor("wv", [D, 256], bf16, kind="ExternalInput")
    wo = nc.dram_tensor("wo", [256, D], f16, kind="ExternalInput")
    outT = nc.dram_tensor("outT", [D, N], f16, kind="ExternalOutput")
    dbg_t = {}
    if dbg:
        for nm, shp, dt in [
            ("d_qh", [128, NZ], bf16),
            ("d_ql", [128, NZ], bf16),
            ("d_kh", [128, NZ], bf16),
            ("d_th", [128, 2 * TW], bf16),
            ("d_V", [128, B * 16 * 128], f16),
            ("d_L15", [128, 2048], f32),
            ("d_P15", [128, 2048], f16),
            ("d_pt15", [128, 16 * 128], f16),
            ("d_attn16", [128, NZ], f16),
        ]:
            dbg_t[nm] = nc.dram_tensor(nm, shp, dt, kind="ExternalOutput")

    # DRAM scratch strips for the diagonal->row shear, one per (z, i) holding
    # BOTH heads: row layout [h0 data W][pad 128][h1 data W][pad 128], pitch
    # P2 = 2W + 256. The pads are poisoned with -1e30 once, outside the timing
    # loop; the sheared read of row ni lands in the pad exactly for the masked
    # (m > n) columns, so causality needs no separate masking pass.
    scr = {}
    for hp in range(2):
        for i in range(16):
            W = 128 * (i + 1)
            P2 = 2 * W + 256
            scr[(hp, i)] = nc.dram_tensor(
                f"scr_{hp}_{i}", [128 * P2], f32, kind="Internal"
            )

    with tile.TileContext(nc) as tc:
        with ExitStack() as ctx:
            wpool = ctx.enter_context(tc.tile_pool(name="wpool", bufs=1))
            big = ctx.enter_context(tc.tile_pool(name="big", bufs=1))
            work = ctx.enter_context(tc.tile_pool(name="work", bufs=3))

            # ---- weights to SBUF ----
            w_sb = {}
            for nm, dram, dt in [
                ("qh", wqh, bf16),
                ("ql", wql, bf16),
                ("kh", wkh, bf16),
                ("kl", wkl, bf16),
                ("v", wv, bf16),
            ]:
                t = wpool.tile([128, KT, 256], dt, tag=f"w{nm}", name=f"w{nm}")
                nc.scalar.dma_start(t[:], dram.ap().rearrange("(t p) m -> p t m", p=128))
                w_sb[nm] = t
            wo_sb = wpool.tile([128, 2, D], f16)
            nc.scalar.dma_start(
                wo_sb[:], wo.ap().rearrange("(g p) d -> p g d", p=128)
            )

            poison = wpool.tile([128, 256], f32)
            nc.vector.memset(poison[:], NEG)
            from concourse.masks import make_identity

            ident16 = wpool.tile([128, 128], f16)
            make_identity(nc, ident16[:])

            # ---- poison pads: once per launch, outside the timing loop ----
            pad_writes = {}
            for hp in range(2):
                for i in range(16):
                    W = 128 * (i + 1)
                    P2 = 2 * W + 256
                    pad_writes[(hp, i)] = nc.scalar.dma_start(
                        AP(scr[(hp, i)], W, [[P2, 128], [W + 128, 2], [1, 128]]),
                        poison[:].rearrange("p (a b) -> p a b", a=2),
                    )

            # ---- persistent activations ----
            # this core owns ONE batch element z and FOUR heads = 2 head-pairs;
            # hp indexes the pair, columns are hp-major: col = hp*N + n
            q_hi = big.tile([128, NZ], bf16)  # [pair-local hd, hp*N+n]
            q_lo = big.tile([128, NZ], bf16)
            k_hi = big.tile([128, NZ], bf16)
            k_lo = big.tile([128, NZ], bf16)
            t_hi = big.tile([128, 2, TW], bf16)  # reversed diag table per pair
            t_lo = big.tile([128, 2, TW], bf16)
            V_all = big.tile([128, 2 * 16, 128], f16)  # [m-part, hp*16+mb, hv]
            attn16 = big.tile([128, NZ], f16)  # [hv, hp*N+n], normalized

            def split_out(pp_t, hi_sb, lo_sb, sl):
                """psum fp32 chunk -> hi (bf16, ACT) and lo = x - hi (bf16, DVE)."""
                nc.scalar.copy(hi_sb[:, sl], pp_t[:])
                nc.vector.tensor_tensor(
                    out=lo_sb[:, sl], in0=pp_t[:], in1=hi_sb[:, sl], op=SUB
                )

            def phase_proj():
                with tc.tile_pool(name="pp", bufs=1, space="PSUM") as pp, tc.tile_pool(
                    name="xp", bufs=3
                ) as xp:

                    # --- q projection: one pass, 2 pair-tiles x 4 chunks ---
                    psq = [
                        pp.tile([128, 512], f32, tag=f"pj{g}{c}", name=f"q{g}{c}")
                        for g in range(2)
                        for c in range(4)
                    ]
                    for t in range(KT):
                        xh_t = xp.tile([128, 2048], bf16, tag="xh")
                        xl_t = xp.tile([128, 2048], bf16, tag="xl")
                        nc.sync.dma_start(
                            xh_t[:],
                            xqh.ap().rearrange("(t p) n -> p t n", p=128)[:, t, :],
                        )
                        nc.sync.dma_start(
                            xl_t[:],
                            xql.ap().rearrange("(t p) n -> p t n", p=128)[:, t, :],
                        )
                        for g in range(2):
                            ws = slice(128 * g, 128 * (g + 1))
                            for c in range(4):
                                rh = xh_t[:, 512 * c : 512 * (c + 1)]
                                rl = xl_t[:, 512 * c : 512 * (c + 1)]
                                ps = psq[4 * g + c]
                                nc.tensor.matmul(
                                    ps[:], w_sb["qh"][:, t, ws], rh,
                                    start=(t == 0), stop=False,
                                )
                                nc.tensor.matmul(
                                    ps[:], w_sb["qh"][:, t, ws], rl,
                                    start=False, stop=False,
                                )
                                nc.tensor.matmul(
                                    ps[:], w_sb["ql"][:, t, ws], rh,
                                    start=False, stop=(t == KT - 1),
                                )
                    for g in range(2):
                        for c in range(4):
                            split_out(
                                psq[4 * g + c], q_hi, q_lo,
                                slice(g * N + 512 * c, g * N + 512 * (c + 1)),
                            )

                    # table loads: after the weight/q traffic is queued, well
                    # before stage_a(0) needs them
                    nc.scalar.dma_start(
                        t_hi[:].rearrange("p a b -> p (a b)"), tbh.ap()
                    )
                    nc.scalar.dma_start(
                        t_lo[:].rearrange("p a b -> p (a b)"), tbl.ap()
                    )

                    # --- k+v projection: 1024-col half passes, (2k+2v)x2 chunks ---
                    vT16 = xp.tile([128, NZ], f16, tag="vT")
                    for half in range(2):
                        cs = 1024 * half
                        psk = [
                            pp.tile([128, 512], f32, tag=f"pj{g}{c}", name=f"k{g}{c}")
                            for g in range(2)
                            for c in range(2)
                        ]
                        psv = [
                            pp.tile([128, 512], f32, tag=f"pj{g}{c + 2}", name=f"v{g}{c}")
                            for g in range(2)
                            for c in range(2)
                        ]
                        for t in range(KT):
                            xh_t = xp.tile([128, 1024], bf16, tag="xh2")
                            xl_t = xp.tile([128, 1024], bf16, tag="xl2")
                            nc.sync.dma_start(
                                xh_t[:],
                                xkh.ap().rearrange("(t p) n -> p t n", p=128)[
                                    :, t, cs : cs + 1024
                                ],
                            )
                            nc.sync.dma_start(
                                xl_t[:],
                                xkl.ap().rearrange("(t p) n -> p t n", p=128)[
                                    :, t, cs : cs + 1024
                                ],
                            )
                            for g in range(2):
                                ws = slice(128 * g, 128 * (g + 1))
                                for c in range(2):
                                    rh = xh_t[:, 512 * c : 512 * (c + 1)]
                                    rl = xl_t[:, 512 * c : 512 * (c + 1)]
                                    ps = psk[2 * g + c]
                                    nc.tensor.matmul(
                                        ps[:], w_sb["kh"][:, t, ws], rh,
                                        start=(t == 0), stop=False,
                                    )
                                    nc.tensor.matmul(
                                        ps[:], w_sb["kh"][:, t, ws], rl,
                                        start=False, stop=False,
                                    )
                                    nc.tensor.matmul(
                                        ps[:], w_sb["kl"][:, t, ws], rh,
                                        start=False, stop=(t == KT - 1),
                                    )
                                    nc.tensor.matmul(
                                        psv[2 * g + c][:], w_sb["v"][:, t, ws], rh,
                                        start=(t == 0), stop=(t == KT - 1),
                                    )
                        for g in range(2):
                            for c in range(2):
                                sl = slice(
                                    g * N + cs + 512 * c, g * N + cs + 512 * (c + 1)
                                )
                                split_out(psk[2 * g + c], k_hi, k_lo, sl)
                                nc.scalar.copy(vT16[:, sl], psv[2 * g + c][:])


                    # --- V tiles [m, hv] via one DMA XBAR transpose ---
                    nc.sync.dma_start_transpose(
                        V_all[:].rearrange("p a b -> p (a b)").rearrange(
                            "p (a b) -> p a b", b=128
                        ),
                        vT16[:],
                    )

            def phase_attn():
                # Software-pipelined over the 32 (z, i) blocks, depth 5:
                #   A(b):   diag matmuls -> per-chunk tT copies (ACT) + strip writes
                #   R(b):   sheared strip read (one 3D-AP DMA, both heads)
                #   B(b):   content matmuls -> in-place add (DVE) -> rowmax
                #   C1(b):  exp (ACT, accum sums) -> recip -> normalize -> ptT DMA
                #   C2(b):  column-packed AV -> attn16 copy; out-proj every 4th
                # Per-engine instruction streams execute in order, so emitting
                # A(i) R(i-1) B(i-2) C1(i-3) C2(i-4) gives each DMA round trip
                # a full pipeline iteration to complete.
                blocks = [(hp, i) for i in range(16) for hp in range(2)]
                NB = len(blocks)
                state = {}
                quarter_left = [8, 8, 8, 8]

                def stage_a(b):
                    hp, i = blocks[b]
                    nb = 128 * i
                    W = nb + 128
                    P2 = 2 * W + 256
                    nchunks = (W + 511) // 512
                    st = state[b] = {}
                    q_b = [
                        (q_hi[64 * h : 64 * (h + 1), hp * N + nb : hp * N + W],
                         q_lo[64 * h : 64 * (h + 1), hp * N + nb : hp * N + W])
                        for h in range(2)
                    ]
                    st["q_b"] = q_b
                    s = scr[(hp, i)]
                    w_insts = []
                    for bc in range(nchunks):
                        wdt = min(512, W - 512 * bc)
                        tps = [
                            pck.tile([128, 512], f32, tag="ck", name=f"tps{h}")
                            for h in range(2)
                        ]
                        for h in range(2):
                            hs = slice(64 * h, 64 * (h + 1))
                            th_c = t_hi[hs, hp, TW - W + 512 * bc : TW - W + 512 * bc + wdt]
                            tl_c = t_lo[hs, hp, TW - W + 512 * bc : TW - W + 512 * bc + wdt]
                            qh_b, ql_b = q_b[h]
                            nc.tensor.matmul(
                                tps[h][:, :wdt], qh_b, th_c, start=True, stop=False
                            )
                            nc.tensor.matmul(
                                tps[h][:, :wdt], qh_b, tl_c, start=False, stop=False
                            )
                            nc.tensor.matmul(
                                tps[h][:, :wdt], ql_b, th_c, start=False, stop=True
                            )
                        tTc = tpo.tile([128, 2, 512], f32, tag="tTc", name="tTc")
                        for h in range(2):
                            if (bc + h) % 3 == 0:
                                nc.scalar.copy(tTc[:, h, :wdt], tps[h][:, :wdt])
                            else:
                                nc.vector.tensor_copy(tTc[:, h, :wdt], tps[h][:, :wdt])
                        if not ABL_NOSHEAR:
                            eng = nc.scalar if bc % 2 == 0 else nc.sync
                            w_insts.append(
                                eng.dma_start(
                                    AP(s, 512 * bc, [[P2, 128], [W + 128, 2], [1, wdt]]),
                                    tTc[:, :, :wdt],
                                )
                            )
                        else:
                            sink = work.tile([128, 1], f32, tag="sink", name="sink")
                            nc.vector.reduce_max(out=sink[:], in_=tTc[:, 0, :4], axis=AX)
                    st["w_insts"] = w_insts

                def stage_r(b):
                    hp, i = blocks[b]
                    W = 128 * (i + 1)
                    P2 = 2 * W + 256
                    st = state[b]
                    s = scr[(hp, i)]
                    Lp2 = lpo.tile([128, 2, 2048], f32, tag="Lp2", name="Lp2")
                    if ABL_NOSHEAR:
                        nc.vector.memset(Lp2[:, :, :W], -1.0)
                    else:
                        nch = (W + 511) // 512
                        ranges = [
                            (512 * c, min(512 * (c + 1), W)) for c in range(nch)
                        ]
                        for c0, c1 in ranges:
                            r_inst = nc.sync.dma_start(
                                Lp2[:, :, c0:c1],
                                AP(
                                    s,
                                    127 + c0,
                                    [[P2 - 1, 128], [W + 128, 2], [1, c1 - c0]],
                                ),
                            )
                            # depend only on the strip writes covering [c0, c1)
                            for wc, w in enumerate(st["w_insts"]):
                                if 512 * wc < c1 and 512 * (wc + 1) > c0:
                                    add_dep_helper(
                                        r_inst.ins, w.ins,
                                        reason="shear read after strip write",
                                    )
                            add_dep_helper(
                                r_inst.ins, pad_writes[(hp, i)].ins,
                                reason="shear read after poison pad",
                            )
                    st["Lp2"] = Lp2

                def stage_b(b, half):
                    hp, i = blocks[b]
                    W = 128 * (i + 1)
                    npair = (W + 1023) // 1024
                    st = state[b]
                    Lp2 = st["Lp2"]
                    q_b = st["q_b"]
                    plo = 0 if half == 0 else (npair + 1) // 2
                    phi = (npair + 1) // 2 if half == 0 else npair
                    for p_ in range(plo, phi):
                        pw = min(1024, W - 1024 * p_)
                        cps = [
                            pbk.tile([128, 1024], f32, tag="bk", name=f"cps{h}")
                            for h in range(2)
                        ]
                        for h in range(2):
                            hs = slice(64 * h, 64 * (h + 1))
                            qh_b, ql_b = q_b[h]
                            for cc in range(0, pw, 512):
                                wdt = min(512, pw - cc)
                                c0 = 1024 * p_ + cc
                                kh_c = k_hi[hs, hp * N + c0 : hp * N + c0 + wdt]
                                kl_c = k_lo[hs, hp * N + c0 : hp * N + c0 + wdt]
                                dstp = cps[h][:, cc : cc + wdt]
                                nc.tensor.matmul(
                                    dstp, qh_b, kh_c, start=True, stop=False,
                                    skip_group_check=True,
                                )
                                nc.tensor.matmul(
                                    dstp, qh_b, kl_c, start=False, stop=False,
                                    skip_group_check=True,
                                )
                                nc.tensor.matmul(
                                    dstp, ql_b, kh_c, start=False, stop=True,
                                    skip_group_check=True,
                                )
                        for h in range(2):
                            sl = Lp2[:, h, 1024 * p_ : 1024 * p_ + pw]
                            nc.vector.tensor_add(sl, cps[h][:, :pw], sl)
                    if half == 1:
                        negmax = work.tile([128, 2], f32, tag="ngm", name="ngm")
                        for h in range(2):
                            nc.vector.tensor_reduce(
                                out=negmax[:, h : h + 1],
                                in_=Lp2[:, h, :W],
                                axis=AX,
                                op=MAX,
                                negate=True,
                            )
                        st["negmax"] = negmax

                def stage_c1(b):
                    hp, i = blocks[b]
                    W = 128 * (i + 1)
                    st = state[b]
                    Lp2 = st["Lp2"]
                    negmax = st["negmax"]
                    P2_sb = ppo.tile([128, 4096], f16, tag="P2", name="P2")
                    ssum = work.tile([128, 2], f32, tag="ssum", name="ssum")
                    for h in range(2):
                        nc.scalar.activation(
                            P2_sb[:, W * h : W * (h + 1)],
                            Lp2[:, h, :W],
                            EXP,
                            bias=negmax[:, h : h + 1],
                            scale=1.0,
                            accum_out=ssum[:, h : h + 1],
                        )
                    rsum = work.tile([128, 2], f32, tag="rsum", name="rsum")
                    nc.vector.reciprocal(rsum[:], ssum[:])
                    pt2 = pto.tile([128, 32, 128], f16, tag="pt2", name="pt2")
                    for h in range(2):
                        nc.vector.tensor_scalar_mul(
                            P2_sb[:, W * h : W * (h + 1)],
                            P2_sb[:, W * h : W * (h + 1)],
                            rsum[:, h : h + 1],
                        )
                    ntile = 2 * (i + 1)
                    for g0 in range(0, ntile, 4):
                        gn = min(4, ntile - g0)
                        tp_ps = pck.tile([128, 512], f16, tag="ck", name="tp_ps")
                        for j in range(gn):
                            nc.tensor.transpose(
                                tp_ps[:, 128 * j : 128 * (j + 1)],
                                P2_sb[:, 128 * (g0 + j) : 128 * (g0 + j + 1)],
                                ident16[:],
                            )
                        dst = pt2[:, g0 : g0 + gn, :].rearrange("p a b -> p (a b)")
                        if (g0 // 4) % 3 == 0:
                            nc.vector.tensor_copy(dst, tp_ps[:, : 128 * gn])
                        else:
                            nc.scalar.copy(dst, tp_ps[:, : 128 * gn])
                    st["pt2"] = pt2
                    if dbg and hp == 0 and i == 15:
                        nc.sync.dma_start(dbg_t["d_L15"].ap(), Lp2[:, 0:1, :].rearrange("p a b -> p (a b)"))
                        nc.sync.dma_start(dbg_t["d_P15"].ap(), P2_sb[:, :2048])
                        nc.sync.dma_start(
                            dbg_t["d_pt15"].ap(),
                            pt2[:, :16, :].rearrange("p a b -> p (a b)"),
                        )

                def stage_c2(b, pa1, oso):
                    hp, i = blocks[b]
                    nb = 128 * i
                    st = state.pop(b)
                    pt2 = st["pt2"]
                    at2 = pa1.tile([128, 128], f32, tag="at2", name="at2")
                    for h in range(2):
                        hs = slice(64 * h, 64 * (h + 1))
                        for mt in range(i + 1):
                            nc.tensor.matmul(
                                at2[hs, :],
                                V_all[:, hp * 16 + mt, hs],
                                pt2[:, (i + 1) * h + mt, :],
                                start=(mt == 0),
                                stop=(mt == i),
                                skip_group_check=True,
                            )
                    nc.vector.tensor_copy(
                        attn16[:, hp * N + nb : hp * N + nb + 128], at2[:]
                    )
                    # out-projection for an n-quarter once all 8 of its blocks
                    # (4 i-values x 2 hp) have passed AV
                    quarter_left[i // 4] -= 1
                    if quarter_left[i // 4] == 0:
                        nn = i // 4
                        for dh in range(2):
                            o_sb = oso.tile([128, 4, 512], f16, tag="osb", name="osb")
                            for dq in range(4):
                                dc = 4 * dh + dq
                                o_ps = pck.tile([128, 512], f32, tag="ck", name="ops")
                                for g in range(2):
                                    nc.tensor.matmul(
                                        o_ps[:],
                                        wo_sb[:, g, 128 * dc : 128 * (dc + 1)],
                                        attn16[:, g * N + 512 * nn : g * N + 512 * (nn + 1)],
                                        start=(g == 0),
                                        stop=(g == 1),
                                    )
                                if dc % 2 == 0:
                                    nc.scalar.copy(o_sb[:, dq, :], o_ps[:])
                                else:
                                    nc.vector.tensor_copy(o_sb[:, dq, :], o_ps[:])
                            nc.sync.dma_start(
                                AP(
                                    outT,
                                    512 * nn + dh * 4 * 128 * N,
                                    [[N, 128], [128 * N, 4], [1, 512]],
                                ),
                                o_sb[:],
                            )

                with tc.tile_pool(name="pa1", bufs=1, space="PSUM") as pa1, tc.tile_pool(
                    name="pbk", bufs=2, space="PSUM"
                ) as pbk, tc.tile_pool(
                    name="pck", bufs=3, space="PSUM"
                ) as pck, tc.tile_pool(name="tpo", bufs=3) as tpo, tc.tile_pool(
                    name="lpo", bufs=4
                ) as lpo, tc.tile_pool(name="ppo", bufs=2) as ppo, tc.tile_pool(
                    name="pto", bufs=2
                ) as pto, tc.tile_pool(name="oso", bufs=2) as oso:
                    if dbg:
                        nc.sync.dma_start(dbg_t["d_qh"].ap(), q_hi[:])
                        nc.sync.dma_start(dbg_t["d_ql"].ap(), q_lo[:])
                        nc.sync.dma_start(dbg_t["d_kh"].ap(), k_hi[:])
                        nc.sync.dma_start(dbg_t["d_th"].ap(), t_hi[:].rearrange("p a b -> p (a b)"))
                        nc.sync.dma_start(
                            dbg_t["d_V"].ap(), V_all[:].rearrange("p a b -> p (a b)")
                        )
                    for idx in range(NB + 6):
                        if 1 <= idx < NB + 1:
                            stage_r(idx - 1)
                        if 6 <= idx:
                            stage_c2(idx - 6, pa1, oso)
                        if 5 <= idx < NB + 5:
                            stage_c1(idx - 5)
                        if 4 <= idx < NB + 4:
                            stage_b(idx - 4, 1)
                        if 3 <= idx < NB + 3:
                            stage_b(idx - 3, 0)
                        if idx < NB:
                            stage_a(idx)
                    if dbg:
                        nc.sync.dma_start(dbg_t["d_attn16"].ap(), attn16[:])

            def loop_body():
                phase_proj()
                phase_attn()

            if loop == 1:
                loop_body()
            else:
                with tc.For_i(0, loop, 1):
                    loop_body()

    nc.compile()
    return nc


def _prep_inputs(x_q, x_kv, to_q, to_kv, for_pos_enc, to_o):
    # core c owns batch element z = c // 4 and heads [4*(c%4), 4*(c%4)+4)
    xq = np.asarray(x_q, dtype=np.float32).transpose(2, 1, 0)  # [D, B, N]
    xkv = np.asarray(x_kv, dtype=np.float32).transpose(2, 1, 0)
    xq_s = [_split_bf16(np.ascontiguousarray(xq[:, z, :])) for z in range(B)]
    xk_s = [_split_bf16(np.ascontiguousarray(xkv[:, z, :])) for z in range(B)]
    if "sctr" not in _cache:
        _cache["sctr"] = _sincos_rev()  # [D, TW] fp32, reversed columns
    sctr = _cache["sctr"]
    to_q = np.asarray(to_q, dtype=np.float32)
    to_kv = np.asarray(to_kv, dtype=np.float32)
    fpe = np.asarray(for_pos_enc, dtype=np.float32)
    to_o = np.asarray(to_o, dtype=np.float32)
    # host-precomputed position table (pure function of the weights):
    # tb_all[h*DQK+d, j] = fpe[h, d, :] . sctr[:, j]
    tb_all = fpe.reshape(H * DQK, D) @ sctr  # [1024, TW] fp32
    in_maps = []
    for c in range(NCORES):
        z, hg = c // 4, c % 4
        hs = slice(4 * hg, 4 * (hg + 1))
        wq = np.ascontiguousarray(to_q[hs].reshape(4 * DQK, D).T)
        wk = np.ascontiguousarray(to_kv[hs, :DQK].reshape(4 * DQK, D).T)
        wvc = np.ascontiguousarray(to_kv[hs, DQK:].reshape(4 * DV, D).T)
        woc = np.ascontiguousarray(to_o[:, hs, :].reshape(D, 4 * DV).T).astype(
            np.float16
        )
        wq_h, wq_l = _split_bf16(wq)
        wk_h, wk_l = _split_bf16(wk)
        wv_h, _ = _split_bf16(wvc)
        # per-core table in the device tile layout [p=pair-local hd, g=pair, j]
        t_core = tb_all[256 * hg : 256 * (hg + 1)].reshape(2, 128, TW)
        t_core = np.ascontiguousarray(t_core.transpose(1, 0, 2).reshape(128, 2 * TW))
        tb_h, tb_l = _split_bf16(t_core)
        in_maps.append(
            {
                "xqh": xq_s[z][0], "xql": xq_s[z][1],
                "xkh": xk_s[z][0], "xkl": xk_s[z][1],
                "tbh": tb_h, "tbl": tb_l,
                "wqh": wq_h, "wql": wq_l,
                "wkh": wk_h, "wkl": wk_l,
                "wv": wv_h, "wo": woc,
            }
        )
    return in_maps


def kernel(x_q, x_kv, to_q, to_kv, for_pos_enc, to_o):
    from concourse.bass_utils import run_bass_kernel_spmd

    if "nc" not in _cache:
        _cache["nc"] = _build()
    nc = _cache["nc"]
    in_maps = _prep_inputs(x_q, x_kv, to_q, to_kv, for_pos_enc, to_o)
    res = run_bass_kernel_spmd(nc, in_maps, core_ids=list(range(NCORES)))
    acc = np.zeros((B, D, N), dtype=np.float32)
    for c in range(NCORES):
        acc[c // 4] += res.results[c]["outT"].astype(np.float32)
    return np.ascontiguousarray(acc.transpose(2, 0, 1)).astype(np.float32)



# revision 114
# speedup vs baseline: 5.2379x; 1.0082x over previous
# Trainium2 Bass kernel for relative-position causal attention
# (Transformer-XL style: logits = q·k + q·table[n-m], causal softmax, AV, out-proj).
#
# Sharding: tensor-parallel over heads — 16 heads / 8 cores = 2 heads per core.
# Each core computes its heads' projections, attention, and a partial output
# projection [B,D,N] fp16; the host sums the 8 partials in fp32.
#
# Position logits are computed in *diagonal* layout (T[ni, j] = q[nb+ni]·table[j],
# a plain matmul since the table index is the diagonal n-m), then converted to
# row layout with a DMA "shear" through a DRAM scratch strip: partition ni reads
# flat offset ni*(P-1) + c with row pitch P = W+128, which is exactly the
# per-partition-shifted gather no on-chip engine can do. The 128-column pad of
# each strip row is pre-poisoned with -1e30, so the causal mask falls out of the
# pitch arithmetic for free (row ni's reads beyond column n land in the pad).
#
# Precision: every logit-affecting matmul runs as a 3-term bf16 hi/lo split
# (a·b ≈ ah·bh + ah·bl + al·bh, error ~2^-16 relative) instead of true fp32 —
# same accuracy class for this problem (|logit|~8e4 needs absolute error << 1),
# but 3 PE cycles/row instead of 4. fp32r (12.6 effective bits measured on HW)
# is too coarse for the logits. The value path (v, softmax weights, AV,
# out-proj) runs in fp16. The V transpose goes through the DMA XBAR; the
# softmax-weight (P) transposes run on the PE (fp16 identity-matmul into an
# fp16 PSUM staging tile, drained by DVE copies) — the XBAR path cost ~65us
# of serialized DMA-engine time that the PE absorbs for ~29us.
#
# Scheduling (tuned against the TimelineSim cost model, which tracked HW
# within 2% on this kernel):
#  - attention pipeline offsets A(idx) R(-1) B0(-3) B1(-4) C1(-5) C2(-6):
#    the shear read gets 2 slots of DMA lead time, and each block's content
#    adds are split across two slots so the DVE add+rowmax burst of one block
#    doesn't convoy the PE behind a single drain.
#  - the shear round trip is chunk-granular end to end: each 512-col strip
#    READ depends only on the strip WRITE covering its columns, so the
#    diag-matmul -> write -> read -> add chain pipelines at 512-col depth
#    instead of whole-block depth.
#  - the position table (a pure function of for_pos_enc and the fixed sincos
#    basis, no activations) is precomputed on the host like any other weight
#    transform and just DMA-loaded, removing the on-device table projection
#    (~41us PE + 8.4MB of sincos traffic).
#  - PSUM banks (8): A-chunk/transpose/out-proj psums rotate through 3x512
#    slots ("ck"); the content-logit psums use 2x1024 double-banked tiles
#    ("bk") so each DVE add covers 1024 columns in one op (half the psum
#    access-latency count); at2 single-buffered.
#  - DMA queue discipline: strip WRITES + pt2 transposes + pad poisons on the
#    Activation DGE queue, strip READS alone on the SP queue, so the sheared
#    reads never sit behind multi-us writes (worth ~25us).
#  - stages are emitted oldest-first inside each slot to cut PE head-of-line
#    blocking; weight loads go on the scalar queue so the first x tiles on SP
#    aren't serialized behind 4MB of weights at kernel start.

from contextlib import ExitStack

import numpy as np

N = 2048
M = 2048
B = 2
D = 1024
H = 16
DQK = 64
DV = 64
NCORES = 8
HPC = H // NCORES  # heads per core = 2
NZ = N * B
KT = D // 128  # 8 contraction tiles
TW = 2048  # table width (diagonals 0..2047)
NEG = -1.0e30

_cache = {}


def _split_bf16(a):
    """hi = bf16(a), lo = bf16(a - hi), as uint16-viewed bfloat16 arrays."""
    import ml_dtypes

    a = np.asarray(a, dtype=np.float32)
    hi = a.astype(ml_dtypes.bfloat16)
    lo = (a - hi.astype(np.float32)).astype(ml_dtypes.bfloat16)
    return np.ascontiguousarray(hi), np.ascontiguousarray(lo)


def _sincos_rev():
    """sincos basis for diagonals d=0..2047, column-reversed, transposed to
    [D, TW] so sctr[:, jr] = sincos(d=TW-1-jr).  Computed with jax on CPU to
    match the reference's fp32 rounding of inv_freq/phases/sin bitwise."""
    try:
        import jax
        import jax.numpy as jnp

        cpu = jax.devices("cpu")[0]
        with jax.default_device(cpu):
            r = jnp.arange(0.0, float(TW), dtype=jnp.float32)
            inv_freq = 1.0 / (
                10000.0 ** (jnp.arange(0.0, D, 2.0, dtype=jnp.float32) / D)
            )
            phases = r[:, None] * inv_freq[None, :]
            sincos = jnp.concatenate([jnp.sin(phases), jnp.cos(phases)], axis=-1)
            sc = np.asarray(sincos)  # [TW, D]
    except Exception:
        r = np.arange(0.0, float(TW), dtype=np.float32)
        inv_freq = (
            1.0
            / (10000.0 ** (np.arange(0.0, D, 2.0, dtype=np.float32) / np.float32(D)))
        ).astype(np.float32)
        phases = (r[:, None] * inv_freq[None, :]).astype(np.float32)
        sc = np.concatenate(
            [np.sin(phases, dtype=np.float32), np.cos(phases, dtype=np.float32)],
            axis=-1,
        )
    return np.ascontiguousarray(sc[::-1].T.astype(np.float32))  # [D, TW]


USE_DMA_T = True   # DMA XBAR transposes for P and V (else PE transposes)
ABL_NOSHEAR = False  # timing ablation: skip strip write+read (wrong results)
USE_TTR = False    # fused TTR crashes the deployed runtime; use add+reduce_max
SKIP_ATTN = False  # debug: bypass phase_attn (attn16 <- memset)
SKIP_AV = False    # debug: run attn up to ptT, skip AV matmuls + attn copy
SKIP_SHEAR = False # debug: skip strip write/read; Lp <- memset
STOP_AT = 9        # debug: 1=diag only, 2=+content/TTR, 3=+exp/mul, 4=+transpose, 9=full
SEQ_HEADS = False  # debug: serialize heads, baseline-style per-head AV psum


def _build(loop=1, dbg=False):
    import concourse.bacc as bacc
    import concourse.mybir as mybir
    import concourse.tile as tile
    from concourse.bass import AP
    from concourse.tile_rust import add_dep_helper

    f32 = mybir.dt.float32
    f16 = mybir.dt.float16
    bf16 = mybir.dt.bfloat16
    AX = mybir.AxisListType.X
    MAX = mybir.AluOpType.max
    ADD = mybir.AluOpType.add
    SUB = mybir.AluOpType.subtract
    EXP = mybir.ActivationFunctionType.Exp

    nc = bacc.Bacc("TRN2", target_bir_lowering=False, debug=False, num_devices=NCORES)

    xqh = nc.dram_tensor("xqh", [D, N], bf16, kind="ExternalInput")
    xql = nc.dram_tensor("xql", [D, N], bf16, kind="ExternalInput")
    xkh = nc.dram_tensor("xkh", [D, N], bf16, kind="ExternalInput")
    xkl = nc.dram_tensor("xkl", [D, N], bf16, kind="ExternalInput")
    tbh = nc.dram_tensor("tbh", [128, 2 * TW], bf16, kind="ExternalInput")
    tbl = nc.dram_tensor("tbl", [128, 2 * TW], bf16, kind="ExternalInput")
    wqh = nc.dram_tensor("wqh", [D, 256], bf16, kind="ExternalInput")
    wql = nc.dram_tensor("wql", [D, 256], bf16, kind="ExternalInput")
    wkh = nc.dram_tensor("wkh", [D, 256], bf16, kind="ExternalInput")
    wkl = nc.dram_tensor("wkl", [D, 256], bf16, kind="ExternalInput")
    wv = nc.dram_tensor("wv", [D, 256], bf16, kind="ExternalInput")
    wo = nc.dram_tensor("wo", [256, D], f16, kind="ExternalInput")
    outT = nc.dram_tensor("outT", [D, N], f16, kind="ExternalOutput")
    dbg_t = {}
    if dbg:
        for nm, shp, dt in [
            ("d_qh", [128, NZ], bf16),
            ("d_ql", [128, NZ], bf16),
            ("d_kh", [128, NZ], bf16),
            ("d_th", [128, 2 * TW], bf16),
            ("d_V", [128, B * 16 * 128], f16),
            ("d_L15", [128, 2048], f32),
            ("d_P15", [128, 2048], f16),
            ("d_pt15", [128, 16 * 128], f16),
            ("d_attn16", [128, NZ], f16),
        ]:
            dbg_t[nm] = nc.dram_tensor(nm, shp, dt, kind="ExternalOutput")

    # DRAM scratch strips for the diagonal->row shear, one per (z, i) holding
    # BOTH heads: row layout [h0 data W][pad 128][h1 data W][pad 128], pitch
    # P2 = 2W + 256. The pads are poisoned with -1e30 once, outside the timing
    # loop; the sheared read of row ni lands in the pad exactly for the masked
    # (m > n) columns, so causality needs no separate masking pass.
    scr = {}
    for hp in range(2):
        for i in range(16):
            W = 128 * (i + 1)
            P2 = 2 * W + 256
            scr[(hp, i)] = nc.dram_tensor(
                f"scr_{hp}_{i}", [128 * P2], f32, kind="Internal"
            )

    with tile.TileContext(nc) as tc:
        with ExitStack() as ctx:
            wpool = ctx.enter_context(tc.tile_pool(name="wpool", bufs=1))
            big = ctx.enter_context(tc.tile_pool(name="big", bufs=1))
            work = ctx.enter_context(tc.tile_pool(name="work", bufs=3))

            # ---- weights to SBUF ----
            w_sb = {}
            for nm, dram, dt in [
                ("qh", wqh, bf16),
                ("ql", wql, bf16),
                ("kh", wkh, bf16),
                ("kl", wkl, bf16),
                ("v", wv, bf16),
            ]:
                t = wpool.tile([128, KT, 256], dt, tag=f"w{nm}", name=f"w{nm}")
                nc.scalar.dma_start(t[:], dram.ap().rearrange("(t p) m -> p t m", p=128))
                w_sb[nm] = t
            wo_sb = wpool.tile([128, 2, D], f16)
            nc.scalar.dma_start(
                wo_sb[:], wo.ap().rearrange("(g p) d -> p g d", p=128)
            )

            poison = wpool.tile([128, 256], f32)
            nc.vector.memset(poison[:], NEG)
            from concourse.masks import make_identity

            ident16 = wpool.tile([128, 128], f16)
            make_identity(nc, ident16[:])

            # ---- poison pads: once per launch, outside the timing loop ----
            pad_writes = {}
            for hp in range(2):
                for i in range(16):
                    W = 128 * (i + 1)
                    P2 = 2 * W + 256
                    pad_writes[(hp, i)] = nc.scalar.dma_start(
                        AP(scr[(hp, i)], W, [[P2, 128], [W + 128, 2], [1, 128]]),
                        poison[:].rearrange("p (a b) -> p a b", a=2),
                    )

            # ---- persistent activations ----
            # this core owns ONE batch element z and FOUR heads = 2 head-pairs;
            # hp indexes the pair, columns are hp-major: col = hp*N + n
            q_hi = big.tile([128, NZ], bf16)  # [pair-local hd, hp*N+n]
            q_lo = big.tile([128, NZ], bf16)
            k_hi = big.tile([128, NZ], bf16)
            k_lo = big.tile([128, NZ], bf16)
            t_hi = big.tile([128, 2, TW], bf16)  # reversed diag table per pair
            t_lo = big.tile([128, 2, TW], bf16)
            V_all = big.tile([128, 2 * 16, 128], f16)  # [m-part, hp*16+mb, hv]
            attn16 = big.tile([128, NZ], f16)  # [hv, hp*N+n], normalized

            def split_out(pp_t, hi_sb, lo_sb, sl):
                """psum fp32 chunk -> hi (bf16, ACT) and lo = x - hi (bf16, DVE)."""
                nc.scalar.copy(hi_sb[:, sl], pp_t[:])
                nc.vector.tensor_tensor(
                    out=lo_sb[:, sl], in0=pp_t[:], in1=hi_sb[:, sl], op=SUB
                )

            def phase_proj():
                with tc.tile_pool(name="pp", bufs=1, space="PSUM") as pp, tc.tile_pool(
                    name="xp", bufs=3
                ) as xp:

                    # --- q projection: one pass, 2 pair-tiles x 4 chunks ---
                    psq = [
                        pp.tile([128, 512], f32, tag=f"pj{g}{c}", name=f"q{g}{c}")
                        for g in range(2)
                        for c in range(4)
                    ]
                    for t in range(KT):
                        xh_t = xp.tile([128, 2048], bf16, tag="xh")
                        xl_t = xp.tile([128, 2048], bf16, tag="xl")
                        nc.sync.dma_start(
                            xh_t[:],
                            xqh.ap().rearrange("(t p) n -> p t n", p=128)[:, t, :],
                        )
                        nc.sync.dma_start(
                            xl_t[:],
                            xql.ap().rearrange("(t p) n -> p t n", p=128)[:, t, :],
                        )
                        for g in range(2):
                            ws = slice(128 * g, 128 * (g + 1))
                            for c in range(4):
                                rh = xh_t[:, 512 * c : 512 * (c + 1)]
                                rl = xl_t[:, 512 * c : 512 * (c + 1)]
                                ps = psq[4 * g + c]
                                nc.tensor.matmul(
                                    ps[:], w_sb["qh"][:, t, ws], rh,
                                    start=(t == 0), stop=False,
                                )
                                nc.tensor.matmul(
                                    ps[:], w_sb["qh"][:, t, ws], rl,
                                    start=False, stop=False,
                                )
                                nc.tensor.matmul(
                                    ps[:], w_sb["ql"][:, t, ws], rh,
                                    start=False, stop=(t == KT - 1),
                                )
                    for g in range(2):
                        for c in range(4):
                            split_out(
                                psq[4 * g + c], q_hi, q_lo,
                                slice(g * N + 512 * c, g * N + 512 * (c + 1)),
                            )

                    # table loads: after the weight/q traffic is queued, well
                    # before stage_a(0) needs them
                    nc.scalar.dma_start(
                        t_hi[:].rearrange("p a b -> p (a b)"), tbh.ap()
                    )
                    nc.scalar.dma_start(
                        t_lo[:].rearrange("p a b -> p (a b)"), tbl.ap()
                    )

                    # --- k+v projection: 1024-col half passes, (2k+2v)x2 chunks ---
                    vT16 = xp.tile([128, NZ], f16, tag="vT")
                    for half in range(2):
                        cs = 1024 * half
                        psk = [
                            pp.tile([128, 512], f32, tag=f"pj{g}{c}", name=f"k{g}{c}")
                            for g in range(2)
                            for c in range(2)
                        ]
                        psv = [
                            pp.tile([128, 512], f32, tag=f"pj{g}{c + 2}", name=f"v{g}{c}")
                            for g in range(2)
                            for c in range(2)
                        ]
                        for t in range(KT):
                            xh_t = xp.tile([128, 1024], bf16, tag="xh2")
                            xl_t = xp.tile([128, 1024], bf16, tag="xl2")
                            nc.sync.dma_start(
                                xh_t[:],
                                xkh.ap().rearrange("(t p) n -> p t n", p=128)[
                                    :, t, cs : cs + 1024
                                ],
                            )
                            nc.sync.dma_start(
                                xl_t[:],
                                xkl.ap().rearrange("(t p) n -> p t n", p=128)[
                                    :, t, cs : cs + 1024
                                ],
                            )
                            for g in range(2):
                                ws = slice(128 * g, 128 * (g + 1))
                                for c in range(2):
                                    rh = xh_t[:, 512 * c : 512 * (c + 1)]
                                    rl = xl_t[:, 512 * c : 512 * (c + 1)]
                                    ps = psk[2 * g + c]
                                    nc.tensor.matmul(
                                        ps[:], w_sb["kh"][:, t, ws], rh,
                                        start=(t == 0), stop=False,
                                    )
                                    nc.tensor.matmul(
                                        ps[:], w_sb["kh"][:, t, ws], rl,
                                        start=False, stop=False,
                                    )
                                    nc.tensor.matmul(
                                        ps[:], w_sb["kl"][:, t, ws], rh,
                                        start=False, stop=(t == KT - 1),
                                    )
                                    nc.tensor.matmul(
                                        psv[2 * g + c][:], w_sb["v"][:, t, ws], rh,
                                        start=(t == 0), stop=(t == KT - 1),
                                    )
                        for g in range(2):
                            for c in range(2):
                                sl = slice(
                                    g * N + cs + 512 * c, g * N + cs + 512 * (c + 1)
                                )
                                split_out(psk[2 * g + c], k_hi, k_lo, sl)
                                nc.scalar.copy(vT16[:, sl], psv[2 * g + c][:])


                    # --- V tiles [m, hv] via one DMA XBAR transpose ---
                    nc.sync.dma_start_transpose(
                        V_all[:].rearrange("p a b -> p (a b)").rearrange(
                            "p (a b) -> p a b", b=128
                        ),
                        vT16[:],
                    )

            def phase_attn():
                # Software-pipelined over the 32 (z, i) blocks, depth 5:
                #   A(b):   diag matmuls -> per-chunk tT copies (ACT) + strip writes
                #   R(b):   sheared strip read (one 3D-AP DMA, both heads)
                #   B(b):   content matmuls -> in-place add (DVE) -> rowmax
                #   C1(b):  exp (ACT, accum sums) -> recip -> normalize -> ptT DMA
                #   C2(b):  column-packed AV -> attn16 copy; out-proj every 4th
                # Per-engine instruction streams execute in order, so emitting
                # A(i) R(i-1) B(i-2) C1(i-3) C2(i-4) gives each DMA round trip
                # a full pipeline iteration to complete.
                blocks = [(hp, i) for i in range(16) for hp in range(2)]
                NB = len(blocks)
                state = {}
                quarter_left = [8, 8, 8, 8]

                def stage_a(b):
                    hp, i = blocks[b]
                    nb = 128 * i
                    W = nb + 128
                    P2 = 2 * W + 256
                    nchunks = (W + 511) // 512
                    st = state[b] = {}
                    q_b = [
                        (q_hi[64 * h : 64 * (h + 1), hp * N + nb : hp * N + W],
                         q_lo[64 * h : 64 * (h + 1), hp * N + nb : hp * N + W])
                        for h in range(2)
                    ]
                    st["q_b"] = q_b
                    s = scr[(hp, i)]
                    w_insts = []
                    for bc in range(nchunks):
                        wdt = min(512, W - 512 * bc)
                        tps = [
                            pck.tile([128, 512], f32, tag="ck", name=f"tps{h}")
                            for h in range(2)
                        ]
                        for h in range(2):
                            hs = slice(64 * h, 64 * (h + 1))
                            th_c = t_hi[hs, hp, TW - W + 512 * bc : TW - W + 512 * bc + wdt]
                            tl_c = t_lo[hs, hp, TW - W + 512 * bc : TW - W + 512 * bc + wdt]
                            qh_b, ql_b = q_b[h]
                            nc.tensor.matmul(
                                tps[h][:, :wdt], qh_b, th_c, start=True, stop=False
                            )
                            nc.tensor.matmul(
                                tps[h][:, :wdt], qh_b, tl_c, start=False, stop=False
                            )
                            nc.tensor.matmul(
                                tps[h][:, :wdt], ql_b, th_c, start=False, stop=True
                            )
                        tTc = tpo.tile([128, 2, 512], f32, tag="tTc", name="tTc")
                        for h in range(2):
                            if (bc + h) % 2 == 0:
                                nc.scalar.copy(tTc[:, h, :wdt], tps[h][:, :wdt])
                            else:
                                nc.vector.tensor_copy(tTc[:, h, :wdt], tps[h][:, :wdt])
                        if not ABL_NOSHEAR:
                            eng = nc.scalar
                            w_insts.append(
                                eng.dma_start(
                                    AP(s, 512 * bc, [[P2, 128], [W + 128, 2], [1, wdt]]),
                                    tTc[:, :, :wdt],
                                )
                            )
                        else:
                            sink = work.tile([128, 1], f32, tag="sink", name="sink")
                            nc.vector.reduce_max(out=sink[:], in_=tTc[:, 0, :4], axis=AX)
                    st["w_insts"] = w_insts

                def stage_r(b):
                    hp, i = blocks[b]
                    W = 128 * (i + 1)
                    P2 = 2 * W + 256
                    st = state[b]
                    s = scr[(hp, i)]
                    Lp2 = lpo.tile([128, 2, 2048], f32, tag="Lp2", name="Lp2")
                    if ABL_NOSHEAR:
                        nc.vector.memset(Lp2[:, :, :W], -1.0)
                    else:
                        nch = (W + 511) // 512
                        ranges = [
                            (512 * c, min(512 * (c + 1), W)) for c in range(nch)
                        ]
                        for c0, c1 in ranges:
                            r_inst = nc.sync.dma_start(
                                Lp2[:, :, c0:c1],
                                AP(
                                    s,
                                    127 + c0,
                                    [[P2 - 1, 128], [W + 128, 2], [1, c1 - c0]],
                                ),
                            )
                            # depend only on the strip writes covering [c0, c1)
                            for wc, w in enumerate(st["w_insts"]):
                                if 512 * wc < c1 and 512 * (wc + 1) > c0:
                                    add_dep_helper(
                                        r_inst.ins, w.ins,
                                        reason="shear read after strip write",
                                    )
                            add_dep_helper(
                                r_inst.ins, pad_writes[(hp, i)].ins,
                                reason="shear read after poison pad",
                            )
                    st["Lp2"] = Lp2

                def stage_b(b, half):
                    hp, i = blocks[b]
                    W = 128 * (i + 1)
                    npair = (W + 1023) // 1024
                    st = state[b]
                    Lp2 = st["Lp2"]
                    q_b = st["q_b"]
                    plo = 0 if half == 0 else (npair + 1) // 2
                    phi = (npair + 1) // 2 if half == 0 else npair
                    for p_ in range(plo, phi):
                        pw = min(1024, W - 1024 * p_)
                        cps = [
                            pbk.tile([128, 1024], f32, tag="bk", name=f"cps{h}")
                            for h in range(2)
                        ]
                        for h in range(2):
                            hs = slice(64 * h, 64 * (h + 1))
                            qh_b, ql_b = q_b[h]
                            for cc in range(0, pw, 512):
                                wdt = min(512, pw - cc)
                                c0 = 1024 * p_ + cc
                                kh_c = k_hi[hs, hp * N + c0 : hp * N + c0 + wdt]
                                kl_c = k_lo[hs, hp * N + c0 : hp * N + c0 + wdt]
                                dstp = cps[h][:, cc : cc + wdt]
                                nc.tensor.matmul(
                                    dstp, qh_b, kh_c, start=True, stop=False,
                                    skip_group_check=True,
                                )
                                nc.tensor.matmul(
                                    dstp, qh_b, kl_c, start=False, stop=False,
                                    skip_group_check=True,
                                )
                                nc.tensor.matmul(
                                    dstp, ql_b, kh_c, start=False, stop=True,
                                    skip_group_check=True,
                                )
                        for h in range(2):
                            sl = Lp2[:, h, 1024 * p_ : 1024 * p_ + pw]
                            nc.vector.tensor_add(sl, cps[h][:, :pw], sl)
                    if half == 1:
                        negmax = work.tile([128, 2], f32, tag="ngm", name="ngm")
                        for h in range(2):
                            nc.vector.tensor_reduce(
                                out=negmax[:, h : h + 1],
                                in_=Lp2[:, h, :W],
                                axis=AX,
                                op=MAX,
                                negate=True,
                            )
                        st["negmax"] = negmax

                def stage_c1(b):
                    hp, i = blocks[b]
                    W = 128 * (i + 1)
                    st = state[b]
                    Lp2 = st["Lp2"]
                    negmax = st["negmax"]
                    P2_sb = ppo.tile([128, 4096], f16, tag="P2", name="P2")
                    ssum = work.tile([128, 2], f32, tag="ssum", name="ssum")
                    for h in range(2):
                        nc.scalar.activation(
                            P2_sb[:, W * h : W * (h + 1)],
                            Lp2[:, h, :W],
                            EXP,
                            bias=negmax[:, h : h + 1],
                            scale=1.0,
                            accum_out=ssum[:, h : h + 1],
                        )
                    rsum = work.tile([128, 2], f32, tag="rsum", name="rsum")
                    nc.vector.reciprocal(rsum[:], ssum[:])
                    pt2 = pto.tile([128, 32, 128], f16, tag="pt2", name="pt2")
                    for h in range(2):
                        nc.vector.tensor_scalar_mul(
                            P2_sb[:, W * h : W * (h + 1)],
                            P2_sb[:, W * h : W * (h + 1)],
                            rsum[:, h : h + 1],
                        )
                    ntile = 2 * (i + 1)
                    for g0 in range(0, ntile, 4):
                        gn = min(4, ntile - g0)
                        tp_ps = pck.tile([128, 512], f16, tag="ck", name="tp_ps")
                        for j in range(gn):
                            nc.tensor.transpose(
                                tp_ps[:, 128 * j : 128 * (j + 1)],
                                P2_sb[:, 128 * (g0 + j) : 128 * (g0 + j + 1)],
                                ident16[:],
                            )
                        dst = pt2[:, g0 : g0 + gn, :].rearrange("p a b -> p (a b)")
                        if (g0 // 4) % 3 == 0:
                            nc.vector.tensor_copy(dst, tp_ps[:, : 128 * gn])
                        else:
                            nc.scalar.copy(dst, tp_ps[:, : 128 * gn])
                    st["pt2"] = pt2
                    if dbg and hp == 0 and i == 15:
                        nc.sync.dma_start(dbg_t["d_L15"].ap(), Lp2[:, 0:1, :].rearrange("p a b -> p (a b)"))
                        nc.sync.dma_start(dbg_t["d_P15"].ap(), P2_sb[:, :2048])
                        nc.sync.dma_start(
                            dbg_t["d_pt15"].ap(),
                            pt2[:, :16, :].rearrange("p a b -> p (a b)"),
                        )

                def stage_c2(b, pa1, oso):
                    hp, i = blocks[b]
                    nb = 128 * i
                    st = state.pop(b)
                    pt2 = st["pt2"]
                    at2 = pa1.tile([128, 128], f32, tag="at2", name="at2")
                    for h in range(2):
                        hs = slice(64 * h, 64 * (h + 1))
                        for mt in range(i + 1):
                            nc.tensor.matmul(
                                at2[hs, :],
                                V_all[:, hp * 16 + mt, hs],
                                pt2[:, (i + 1) * h + mt, :],
                                start=(mt == 0),
                                stop=(mt == i),
                                skip_group_check=True,
                            )
                    nc.vector.tensor_copy(
                        attn16[:, hp * N + nb : hp * N + nb + 128], at2[:]
                    )
                    # out-projection for an n-quarter once all 8 of its blocks
                    # (4 i-values x 2 hp) have passed AV
                    quarter_left[i // 4] -= 1
                    if quarter_left[i // 4] == 0:
                        nn = i // 4
                        for dh in range(2):
                            o_sb = oso.tile([128, 4, 512], f16, tag="osb", name="osb")
                            for dq in range(4):
                                dc = 4 * dh + dq
                                o_ps = pck.tile([128, 512], f32, tag="ck", name="ops")
                                for g in range(2):
                                    nc.tensor.matmul(
                                        o_ps[:],
                                        wo_sb[:, g, 128 * dc : 128 * (dc + 1)],
                                        attn16[:, g * N + 512 * nn : g * N + 512 * (nn + 1)],
                                        start=(g == 0),
                                        stop=(g == 1),
                                    )
                                if dc % 2 == 0:
                                    nc.scalar.copy(o_sb[:, dq, :], o_ps[:])
                                else:
                                    nc.vector.tensor_copy(o_sb[:, dq, :], o_ps[:])
                            nc.sync.dma_start(
                                AP(
                                    outT,
                                    512 * nn + dh * 4 * 128 * N,
                                    [[N, 128], [128 * N, 4], [1, 512]],
                                ),
                                o_sb[:],
                            )

                with tc.tile_pool(name="pa1", bufs=1, space="PSUM") as pa1, tc.tile_pool(
                    name="pbk", bufs=2, space="PSUM"
                ) as pbk, tc.tile_pool(
                    name="pck", bufs=3, space="PSUM"
                ) as pck, tc.tile_pool(name="tpo", bufs=3) as tpo, tc.tile_pool(
                    name="lpo", bufs=4
                ) as lpo, tc.tile_pool(name="ppo", bufs=2) as ppo, tc.tile_pool(
                    name="pto", bufs=2
                ) as pto, tc.tile_pool(name="oso", bufs=2) as oso:
                    if dbg:
                        nc.sync.dma_start(dbg_t["d_qh"].ap(), q_hi[:])
                        nc.sync.dma_start(dbg_t["d_ql"].ap(), q_lo[:])
                        nc.sync.dma_start(dbg_t["d_kh"].ap(), k_hi[:])
                        nc.sync.dma_start(dbg_t["d_th"].ap(), t_hi[:].rearrange("p a b -> p (a b)"))
                        nc.sync.dma_start(
                            dbg_t["d_V"].ap(), V_all[:].rearrange("p a b -> p (a b)")
                        )
                    for idx in range(NB + 6):
                        if 1 <= idx < NB + 1:
                            stage_r(idx - 1)
                        if 6 <= idx:
                            stage_c2(idx - 6, pa1, oso)
                        if 5 <= idx < NB + 5:
                            stage_c1(idx - 5)
                        if 4 <= idx < NB + 4:
                            stage_b(idx - 4, 1)
                        if 3 <= idx < NB + 3:
                            stage_b(idx - 3, 0)
                        if idx < NB:
                            stage_a(idx)
                    if dbg:
                        nc.sync.dma_start(dbg_t["d_attn16"].ap(), attn16[:])

            def loop_body():
                phase_proj()
                phase_attn()

            if loop == 1:
                loop_body()
            else:
                with tc.For_i(0, loop, 1):
                    loop_body()

    nc.compile()
    return nc


def _prep_inputs(x_q, x_kv, to_q, to_kv, for_pos_enc, to_o):
    # core c owns batch element z = c // 4 and heads [4*(c%4), 4*(c%4)+4)
    xq = np.asarray(x_q, dtype=np.float32).transpose(2, 1, 0)  # [D, B, N]
    xkv = np.asarray(x_kv, dtype=np.float32).transpose(2, 1, 0)
    xq_s = [_split_bf16(np.ascontiguousarray(xq[:, z, :])) for z in range(B)]
    xk_s = [_split_bf16(np.ascontiguousarray(xkv[:, z, :])) for z in range(B)]
    if "sctr" not in _cache:
        _cache["sctr"] = _sincos_rev()  # [D, TW] fp32, reversed columns
    sctr = _cache["sctr"]
    to_q = np.asarray(to_q, dtype=np.float32)
    to_kv = np.asarray(to_kv, dtype=np.float32)
    fpe = np.asarray(for_pos_enc, dtype=np.float32)
    to_o = np.asarray(to_o, dtype=np.float32)
    # host-precomputed position table (pure function of the weights):
    # tb_all[h*DQK+d, j] = fpe[h, d, :] . sctr[:, j]
    tb_all = fpe.reshape(H * DQK, D) @ sctr  # [1024, TW] fp32
    in_maps = []
    for c in range(NCORES):
        z, hg = c // 4, c % 4
        hs = slice(4 * hg, 4 * (hg + 1))
        wq = np.ascontiguousarray(to_q[hs].reshape(4 * DQK, D).T)
        wk = np.ascontiguousarray(to_kv[hs, :DQK].reshape(4 * DQK, D).T)
        wvc = np.ascontiguousarray(to_kv[hs, DQK:].reshape(4 * DV, D).T)
        woc = np.ascontiguousarray(to_o[:, hs, :].reshape(D, 4 * DV).T).astype(
            np.float16
        )
        wq_h, wq_l = _split_bf16(wq)
        wk_h, wk_l = _split_bf16(wk)
        wv_h, _ = _split_bf16(wvc)
        # per-core table in the device tile layout [p=pair-local hd, g=pair, j]
        t_core = tb_all[256 * hg : 256 * (hg + 1)].reshape(2, 128, TW)
        t_core = np.ascontiguousarray(t_core.transpose(1, 0, 2).reshape(128, 2 * TW))
        tb_h, tb_l = _split_bf16(t_core)
        in_maps.append(
            {
                "xqh": xq_s[z][0], "xql": xq_s[z][1],
                "xkh": xk_s[z][0], "xkl": xk_s[z][1],
                "tbh": tb_h, "tbl": tb_l,
                "wqh": wq_h, "wql": wq_l,
                "wkh": wk_h, "wkl": wk_l,
                "wv": wv_h, "wo": woc,
            }
        )
    return in_maps


def kernel(x_q, x_kv, to_q, to_kv, for_pos_enc, to_o):
    from concourse.bass_utils import run_bass_kernel_spmd

    if "nc" not in _cache:
        _cache["nc"] = _build()
    nc = _cache["nc"]
    in_maps = _prep_inputs(x_q, x_kv, to_q, to_kv, for_pos_enc, to_o)
    res = run_bass_kernel_spmd(nc, in_maps, core_ids=list(range(NCORES)))
    acc = np.zeros((B, D, N), dtype=np.float32)
    for c in range(NCORES):
        acc[c // 4] += res.results[c]["outT"].astype(np.float32)
    return np.ascontiguousarray(acc.transpose(2, 0, 1)).astype(np.float32)

